# revision 8
# baseline (speedup 1.0000x reference)
"""Trainium2 Bass kernel for grouped cumulative-sim causal attention.

Reference computation (B=2, N=2048, G=4 groups, H=8 heads, DH=64):
  q/k/v = 1x1-conv projections of x [B, 2048, N]
  sim[b,g,h] = cumsum_over_g( (SCALE*q) @ k^T )   (the group-cumsum)
  out = softmax(causal(sim)) @ v ; y = Wout @ out + b_out

Sharding: one head h per NeuronCore (8 cores), both batches and all 4
groups local to the core (the cumsum couples g only). Each core computes
a partial y (its head's 256-channel contribution through Wout); the host
sums the 8 partials and adds b_out.

Device-side layout (everything transposed so the softmax j-axis lands on
PSUM partitions and attn comes out ready for the AV matmul):
  q,k  [dh=64(part,2 groups/tile), gpair, i/j] fp16 (Wq pre-scaled)
  sim_T[j(part), i]  ONE PSUM tile per j-tile: the four group products
        p_g = q_g.k_g accumulate SERIALLY (start g0, 3 more K=64 adds);
        exp SNAPSHOTS the tile after each add -> cumsum for free with
        4 matmuls instead of 6 stacked ones, and 1 sim PSUM tile live.
  causal mask  -60000 added ONCE per diag tile right after p0 (the
        snapshot reads all see it; -60000+sim stays << fp16-safe)
  exp   ScalarE PSUM->SBUF fp16, bias=-6 for fp16 range (cancels in
        softmax: atg and the denominator row scale together)
  AV    lhsT = v^T tile augmented with a ones column -> row 64 of
        the PSUM output accumulates the softmax denominator
  1/s   vector reciprocal_approx_fast + broadcast K=1 matmul (f32r)
  y     Wout^T-slice matmul, fp16 partials DMA'd to a tile-packed dram
        layout; the host unpermutes and sums the 8 cores in fp32.
All matmul operands fp16 (rel err ~6e-4 vs the 2e-2 gate); PSUM fp32.
x / weights / y all use host-packed dram layouts so every DMA moves
contiguous 2-4KB runs per partition (descriptor-efficient).
DMA queues: x tiles + yin repack on sync, weights/consts on gpsimd,
y output on scalar.
"""

import numpy as np

B, N = 2, 2048
G, H, DH = 4, 8, 64
CIN = 2048            # input channels  (= DIM*G)
PH = G * DH           # 256 inner channels per head
SCALE = DH ** -0.5
P = 128
FB = 512              # i-block width
NB = N // FB          # 4 i-blocks
CT = CIN // P         # 16 contraction tiles
JT = N // P           # 16 j-tiles
MASKV = -60000.0      # fp16-safe causal mask addend
EBIAS = -6.0          # exp bias: atg = e^(sim-6), cancels in softmax

_cache = {}
USE_STACKED_ATTENTION = True


def _build_program():
    import concourse.bass as bass  # noqa: F401
    import concourse.tile as tile
    from concourse import bacc, mybir

    f32 = mybir.dt.float32
    f32r = mybir.dt.float32r
    f16 = mybir.dt.float16
    Exp = mybir.ActivationFunctionType.Exp

    # Bacc (not raw Bass): its compile() splits multi-wait matmuls
    # (move_matmul_waits_to_ldweights / generate_event_semaphores) —
    # the S3_LW matmul instruction has a single hardware wait slot.
    nc = bacc.Bacc(None, target_bir_lowering=False)
    # x packed on host as [(b,ib,c4), p, (o,f)] so each xt4 load is one
    # contiguous [128, 2048] block (4KB/partition descriptors)
    x_d = nc.dram_tensor("x", [B * NB * 4, P, 4 * FB], f16, kind="ExternalInput")
    wq_d = nc.dram_tensor("wqT", [4, P, 4 * PH], f16, kind="ExternalInput")
    wk_d = nc.dram_tensor("wkT", [4, P, 4 * PH], f16, kind="ExternalInput")
    wv_d = nc.dram_tensor("wvT", [4, P, 4 * PH], f16, kind="ExternalInput")
    wo_d = nc.dram_tensor("woT", [2, P, CIN], f16, kind="ExternalInput")
    tri_d = nc.dram_tensor("tri", [P, P], f16, kind="ExternalInput")
    id_d = nc.dram_tensor("ident", [P, P], f16, kind="ExternalInput")
    on_d = nc.dram_tensor("onesr", [3, 64], f32r, kind="ExternalInput")
    vo_d = nc.dram_tensor("vones", [P, JT, 4], f16, kind="ExternalInput")
    # y packed as [(b,ib,ot4), p, (o,f)] fp16; host unpermutes + sums
    y_d = nc.dram_tensor("y", [B * NB * 4, P, 4 * FB], f16, kind="ExternalOutput")

    with tile.TileContext(nc) as tc:
        from contextlib import ExitStack

        with ExitStack() as ctx:
            consts = ctx.enter_context(tc.tile_pool(name="consts", bufs=1))
            big = ctx.enter_context(tc.tile_pool(name="big", bufs=1))
            xp = ctx.enter_context(tc.tile_pool(name="xp", bufs=9))
            atp = ctx.enter_context(tc.tile_pool(name="atp", bufs=5))
            avsp = ctx.enter_context(tc.tile_pool(name="avsp", bufs=1))
            rcp = ctx.enter_context(tc.tile_pool(name="rcp", bufs=2))
            ysp = ctx.enter_context(tc.tile_pool(name="ysp", bufs=3))
            simp = ctx.enter_context(tc.tile_pool(name="simp", bufs=2, space="PSUM"))
            avp = ctx.enter_context(tc.tile_pool(name="avp", bufs=4, space="PSUM"))
            gp = ctx.enter_context(tc.tile_pool(name="gp", bufs=2, space="PSUM"))

            # ---- static tensors (gpsimd DMA queue; x uses sync, y scalar)
            # small early-needed constants FIRST: the first attention's
            # masking + AV wait on tri/vones
            tri_sb = consts.tile([P, P], f16)
            nc.scalar.dma_start(tri_sb, tri_d[:, :])
            id_sb = consts.tile([P, P], f16)
            nc.scalar.dma_start(id_sb, id_d[:, :])
            # ones rows at partitions 0/32/64 for the K=1 1/s broadcast
            # matmuls (matmul base partition must be 0/32/64)
            on_sb = consts.tile([65, 64], f32r)
            for go in range(3):
                nc.scalar.dma_start(
                    on_sb[32 * go:32 * go + 1, :], on_d[go:go + 1, :]
                )
            # persistent denominator-gather tile: memset once so the
            # full-tile reciprocal never reads uninitialized rows
            den = consts.tile([P, FB], f32)
            nc.gpsimd.memset(den, 1.0)
            # per-partition exp-bias column (activation bias must be an AP)
            ebias = consts.tile([P, 1], f32)
            nc.gpsimd.memset(ebias, EBIAS)
            wq_sb = consts.tile([P, CT, PH], f16)
            wk_sb = consts.tile([P, CT, PH], f16)
            wv_sb = consts.tile([P, CT, PH], f16)
            wo_sb = consts.tile([P, 2, CIN], f16)

            # per-batch state, allocated lazily by phase1(step) so the
            # software pipeline below controls allocation order
            state = {}

            def get_state(b):
                if b not in state:
                    q_sb = big.tile([P, 2, N], f16, tag="q", name=f"q{b}")
                    k_sb = big.tile([P, 2, N], f16, tag="k", name=f"k{b}")
                    v_sb = big.tile([P, JT, 4 * 65], f16, tag="v", name=f"v{b}")
                    yin = big.tile([P, 2, N], f16, tag="yin", name=f"yin{b}")
                    # ones column per group (softmax-denominator row of AV)
                    nc.scalar.dma_start(
                        v_sb.rearrange("p t (g c) -> p t g c", g=4)[:, :, :, 64:65],
                        vo_d[:, :, :].rearrange("p t g -> p t g ()"),
                    )
                    state[b] = (q_sb, k_sb, v_sb, yin)
                return state[b]

            # b=0 state now: its vones DMA precedes the weight bulk
            get_state(0)
            # weight loads in CONSUMPTION order (q chains run first, then
            # k, then v), 4-ct chunks so each chain starts as its slice
            # lands; each chunk is one contiguous [128, 1024] block
            for wsb, wd in ((wq_sb, wq_d), (wk_sb, wk_d), (wv_sb, wv_d)):
                for ch in range(4):
                    nc.scalar.dma_start(
                        wsb[:, 4 * ch:4 * ch + 4, :], wd[ch]
                        .rearrange("p (c m) -> p c m", c=4),
                    )
            for kc in range(2):
                nc.scalar.dma_start(wo_sb[:, kc, :], wo_d[kc])

            def phase1_units(b, ib):
                """Return a list of closures, one dense PE matmul chain each
                (4 q/k row-tiles + 4 v j-tiles); x DMAs are issued eagerly."""
                q_sb, k_sb, v_sb, _ = get_state(b)
                isl = slice(ib * FB, (ib + 1) * FB)
                xts = []
                for c4 in range(4):
                    xt4 = xp.tile([P, 4, FB], f16, tag="xt", name=f"xt{c4}")
                    nc.sync.dma_start(
                        xt4,
                        x_d[(b * NB + ib) * 4 + c4]
                        .rearrange("p (o f) -> p o f", o=4),
                    )
                    for i in range(4):
                        xts.append(xt4[:, i, :])
                units = []

                def qk_unit(dest, wsb, m):
                    def emit():
                        ps = gp.tile([P, FB], f32, tag="gp", name="qkps")
                        for ct in range(CT):
                            nc.tensor.matmul(
                                ps, wsb[:, ct, m * P:(m + 1) * P], xts[ct],
                                start=(ct == 0), stop=(ct == CT - 1),
                            )
                        nc.vector.tensor_copy(dest[:, m, isl], ps)
                    return emit

                def v_unit(jj):
                    def emit():
                        jt = ib * 4 + jj
                        ps = gp.tile([P, PH], f32, tag="gp", name="vps")
                        for ct in range(CT):
                            nc.tensor.matmul(
                                ps, xts[ct][:, jj * P:(jj + 1) * P],
                                wv_sb[:, ct, :],
                                start=(ct == 0), stop=(ct == CT - 1),
                            )
                        nc.vector.tensor_copy(
                            v_sb[:, jt, :]
                            .rearrange("p (g c) -> p g c", g=4)[:, :, 0:64],
                            ps.rearrange("p (g c) -> p g c", g=4),
                        )
                    return emit

                for dest, wsb in ((q_sb, wq_sb), (k_sb, wk_sb)):
                    for m in range(2):
                        units.append(qk_unit(dest, wsb, m))
                for jj in range(4):
                    units.append(v_unit(jj))
                return units

            def yproj_units(b, ib):
                """One closure per output row-tile (2-matmul chain + copy);
                every 4th unit DMAs the batched [512-row] block out."""
                _, _, _, yin = get_state(b)
                isl = slice(ib * FB, (ib + 1) * FB)
                units = []
                ys4_box = [None]

                def y_unit(ot):
                    def emit():
                        yp = gp.tile([P, FB], f32, tag="gp", name="yp")
                        for kc in range(2):
                            nc.tensor.matmul(
                                yp, wo_sb[:, kc, ot * P:(ot + 1) * P],
                                yin[:, kc, isl],
                                start=(kc == 0), stop=(kc == 1),
                            )
                        if ot % 4 == 0:
                            ys4_box[0] = ysp.tile(
                                [P, 4, FB], f16, tag="ys", name="ys"
                            )
                        ys4 = ys4_box[0]
                        nc.vector.tensor_copy(ys4[:, ot % 4, :], yp)
                        if ot % 4 == 3:
                            nc.scalar.dma_start(
                                y_d[(b * NB + ib) * 4 + ot // 4]
                                .rearrange("p (o f) -> p o f", o=4),
                                ys4,
                            )
                    return emit

                for ot in range(CT):
                    units.append(y_unit(ot))
                return units

            def attention_stacked(b, ib, filler, front=True):
                # baseline-structure cumsum (stacked re-contraction, one
                # psum tile per (jt, g)) — bisection fallback
                q_sb, k_sb, v_sb, _ = get_state(b)
                jmax = 4 * (ib + 1)
                avs_t = [
                    avp.tile([65, FB], f32, tag="av", name=f"av{g}")
                    for g in range(4)
                ]
                nfill = len(filler)
                nslot = jmax * 4
                for jt in range(jmax):
                    jsl = slice(jt * P, (jt + 1) * P)
                    diag = jt >= 4 * ib
                    cs = (jt - 4 * ib) * P if diag else 0
                    isl = slice(ib * FB + cs, (ib + 1) * FB)
                    for g in range(4):
                        slot = jt * 4 + g

                        def fsched(u):
                            if not front:
                                return u
                            return min(1.35 * u, 0.45 + 0.55 * u)
                        take = int(nfill * fsched((slot + 1) / nslot)) - \
                            int(nfill * fsched(slot / nslot))
                        for _ in range(take):
                            filler.pop(0)()
                        sim = simp.tile([P, FB], f32, tag="sim", name="sim")
                        full, half = (g + 1) // 2, (g + 1) % 2
                        for t in range(full):
                            nc.tensor.matmul(
                                sim[:, cs:], k_sb[:, t, jsl], q_sb[:, t, isl],
                                start=(t == 0),
                                stop=(t == full - 1 and not half and not diag),
                            )
                        if half:
                            nc.tensor.matmul(
                                sim[:, cs:], k_sb[0:64, full, jsl],
                                q_sb[0:64, full, isl],
                                start=(full == 0), stop=(not diag),
                            )
                        if diag:
                            nc.tensor.matmul(
                                sim[:, cs:cs + P], id_sb, tri_sb,
                                start=False, stop=True,
                            )
                        atg = atp.tile([P, FB], f16, tag="at", name="at")
                        nc.scalar.activation(
                            atg[:, cs:], sim[:, cs:], Exp, bias=ebias[:, :]
                        )
                        nc.tensor.matmul(
                            avs_t[g][:, cs:],
                            v_sb[:, jt, g * 65:(g + 1) * 65],
                            atg[:, cs:],
                            start=(jt == 0),
                            stop=(jt == jmax - 1),
                        )
                return avs_t

            def attention(b, ib, filler, front=True):
                if USE_STACKED_ATTENTION:
                    return attention_stacked(b, ib, filler, front)
                q_sb, k_sb, v_sb, _ = get_state(b)
                jmax = 4 * (ib + 1)
                avs_t = [
                    avp.tile([65, FB], f32, tag="av", name=f"av{g}")
                    for g in range(4)
                ]
                nfill = len(filler)
                nslot = jmax * 4
                for jt in range(jmax):
                    jsl = slice(jt * P, (jt + 1) * P)
                    diag = jt >= 4 * ib
                    # columns < cs of this i-block are fully above the
                    # causal diagonal for this j-tile: skip them outright
                    cs = (jt - 4 * ib) * P if diag else 0
                    isl = slice(ib * FB + cs, (ib + 1) * FB)
                    # ONE sim tile per j-tile: groups accumulate serially,
                    # exp snapshots the running sum -> cumsum for free
                    sim = simp.tile([P, FB], f32, tag="sim", name="sim")
                    for g in range(4):
                        # interleave dense independent matmul chains (next
                        # step's projections, previous step's y tiles)
                        # between the attention chains so the PE stream
                        # stays dense and the p-state stays maxed
                        slot = jt * 4 + g

                        def fsched(u):
                            # denser fillers early (normalize(k) is still
                            # freeing AV banks) but keep a reserve for the
                            # late slots where attention alone is
                            # exp-throughput-limited; y-only fillers (last
                            # step) spread uniformly instead
                            if not front:
                                return u
                            return min(1.35 * u, 0.45 + 0.55 * u)
                        take = int(nfill * fsched((slot + 1) / nslot)) - \
                            int(nfill * fsched(slot / nslot))
                        for _ in range(take):
                            filler.pop(0)()
                        go = 64 * (g % 2)
                        # the psum group stays OPEN until the last add (a
                        # stopped group cannot be accumulated into again);
                        # exp snapshots read the open group's psum freely
                        nc.tensor.matmul(
                            sim[:, cs:],
                            k_sb[go:go + 64, g // 2, jsl],
                            q_sb[go:go + 64, g // 2, isl],
                            start=(g == 0),
                            stop=(not (g == 0 and diag)),
                            skip_group_check=(g != 0),
                        )
                        if g == 0 and diag:
                            # causal mask: -60000 into the 128-wide diag
                            # window ONCE; every later snapshot sees it
                            nc.tensor.matmul(
                                sim[:, cs:cs + P], id_sb, tri_sb,
                                start=False, stop=True,
                            )
                        atg = atp.tile([P, FB], f16, tag="at", name="at")
                        nc.scalar.activation(
                            atg[:, cs:], sim[:, cs:], Exp, bias=ebias[:, :]
                        )
                        nc.tensor.matmul(
                            avs_t[g][:, cs:],
                            v_sb[:, jt, g * 65:(g + 1) * 65],
                            atg[:, cs:],
                            start=(jt == 0),
                            stop=(jt == jmax - 1),
                        )
                return avs_t

            def normalize(b, ib, avs_t):
                _, _, _, yin = get_state(b)
                isl = slice(ib * FB, (ib + 1) * FB)
                avs = avsp.tile([64, 4, FB], f16, tag="avs", name="avs")
                avr = avsp.tile([64, 4, FB], f32, tag="avr", name="avr")
                # copy-first: the denominator row and the raw AV rows leave
                # PSUM immediately so the AV banks free up for the next
                # attention step without waiting on the reciprocal chain
                # (scalar-engine avr copies measured 20us WORSE — they
                # delay the next step's exps; keep all of this on vector)
                for g in range(4):
                    nc.vector.tensor_copy(
                        den[32 * g:32 * g + 1, :], avs_t[g][64:65, :]
                    )
                    nc.vector.tensor_copy(avr[:, g, :], avs_t[g][0:64, :])
                # ONE full-tile fast reciprocal covers all four groups (the
                # garbage rows are memset to 1.0; cost scales with free
                # size, not partitions)
                rcr = rcp.tile([P, FB], f32, tag="rcr", name="rcr")
                nc.vector.reciprocal_approx_fast(rcr, den)
                # bounce the 4 live rows through fp32->f32r copies (the
                # f32r matmul requires pre-rounded operands); bases 0/32/64
                # (matmul base partition cannot be 96, so g=3 sits at 0 of
                # a second tile, emitted late, off the g0-g2 mul path)
                rcq = rcp.tile([65, FB], f32r, tag="rcq", name="rcq")
                rc3 = rcp.tile([1, FB], f32r, tag="rc3", name="rc3")
                with nc.allow_low_precision(
                    reason="1/s rounded to f32r for the broadcast matmul"
                ):
                    for g in range(3):
                        nc.vector.tensor_copy(
                            rcq[32 * g:32 * g + 1, :], rcr[32 * g:32 * g + 1, :]
                        )
                for g in range(4):
                    if g == 3:
                        with nc.allow_low_precision(
                            reason="1/s rounded to f32r for the broadcast matmul"
                        ):
                            nc.vector.tensor_copy(rc3, rcr[96:97, :])
                    rrow = rc3[0:1, :] if g == 3 \
                        else rcq[32 * g:32 * g + 1, :]
                    orow = on_sb[0:1, :] if g == 3 \
                        else on_sb[32 * g:32 * g + 1, :]
                    bcps = gp.tile([64, FB], f32, tag="gp", name="bcps")
                    nc.tensor.matmul(bcps, orow, rrow, start=True, stop=True)
                    nc.vector.tensor_mul(avs[:, g, :], avr[:, g, :], bcps)
                    # per-group yin repack DMA: fires as soon as this
                    # group's mul lands, instead of waiting for all four
                    nc.sync.dma_start(
                        yin[64 * (g % 2):64 * (g % 2) + 64, g // 2, isl],
                        avs[:, g, :],
                    )

            # software pipeline: attention(k) is emitted with the next
            # step's projection chains and the previous step's y tiles
            # interleaved between its cumsum chains.
            def interleave(a, bu):
                out = []
                la, lb = list(a), list(bu)
                while la or lb:
                    if la:
                        out.append(la.pop(0))
                    if lb:
                        out.append(lb.pop(0))
                return out

            steps = [(b, ib) for b in range(B) for ib in range(NB)]
            for u in phase1_units(*steps[0]):
                u()
            prev_yp = []
            carry = []
            for k, (b, ib) in enumerate(steps):
                nxt = phase1_units(*steps[k + 1]) if k + 1 < len(steps) else []
                if nxt:
                    # late attentions bank a few y units so the LAST
                    # attention (no projection filler left) doesn't run
                    # bare into its exp-throughput limit
                    take = prev_yp[:12] if k >= 5 else prev_yp
                    carry += prev_yp[len(take):]
                    filler = interleave(nxt, take)
                else:
                    filler = interleave(carry, prev_yp)
                avs_t = attention(b, ib, filler, front=bool(nxt))
                normalize(b, ib, avs_t)
                prev_yp = yproj_units(b, ib)
            for u in prev_yp:
                u()
    if not nc.is_finalized():
        nc.finalize()
    return nc


def _host_inputs(x, Wq, Wkv, Wout):
    """Per-core input maps (head h on core h), tile-packed for DMA."""
    import ml_dtypes

    hf = ml_dtypes.float16 if not hasattr(np, "float16") else np.float16
    hf = np.float16
    # mask for the 128-wide diagonal window: j > i (local) -> -60000
    jj = np.arange(P)[:, None]
    ii = np.arange(P)[None, :]
    tri = np.where(jj > ii, MASKV, 0.0).astype(hf)
    ident = np.eye(P, dtype=np.float32).astype(hf)

    # x packed: [(b, ib, c4), p, (o, f)]; channel = c4*512 + o*128 + p
    xp = (
        np.asarray(x, np.float32).astype(hf)
        .reshape(B, 4, 4, P, NB, FB)        # b, c4, o, p, ib, f
        .transpose(0, 4, 1, 3, 2, 5)        # b, ib, c4, p, o, f
        .reshape(B * NB * 4, P, 4 * FB)
    )
    xp = np.ascontiguousarray(xp)
    Wk, Wv = Wkv[:CIN], Wkv[CIN:]
    onesr = np.ones((3, 64), np.float32)

    def pack_w(wt):  # [2048, 256] -> [4, P, 4*PH]; row ct*128+p, col m
        return np.ascontiguousarray(
            wt.reshape(4, 4, P, PH).transpose(0, 2, 1, 3).reshape(4, P, 4 * PH)
        ).astype(hf)

    in_maps = []
    for h in range(H):
        rows = (np.arange(G)[:, None] * (H * DH) + h * DH
                + np.arange(DH)[None, :]).reshape(-1)          # (g d) order
        in_maps.append({
            "x": xp,
            "wqT": pack_w((Wq[rows] * SCALE).T.astype(np.float32)),
            "wkT": pack_w(Wk[rows].T.astype(np.float32)),
            "wvT": pack_w(Wv[rows].T.astype(np.float32)),
            "woT": np.ascontiguousarray(
                Wout[:, rows].T.reshape(2, P, CIN)
            ).astype(hf),
            "tri": tri,
            "ident": ident,
            "onesr": onesr,
            "vones": np.ones((P, JT, 4), hf),
        })
    return in_maps


def _install_profile_hook():
    """Register the NTFF profile hook that the image's antenv lacks, and
    keep profile artifacts local (no bucket upload)."""
    import sys
    import types
    import ctypes
    import contextlib

    if "antenv.axon_hooks" in sys.modules:
        return
    so_path = "/opt/axon/libaxon_pjrt.so"
    lib = ctypes.CDLL(so_path)
    if not hasattr(lib, "axon_start_nrt_profile"):
        raise RuntimeError("libaxon_pjrt.so lacks profiling symbols")
    lib.axon_start_nrt_profile.argtypes = [
        ctypes.POINTER(ctypes.c_int64), ctypes.c_size_t,
    ]
    lib.axon_start_nrt_profile.restype = ctypes.c_int64
    lib.axon_stop_nrt_profile.argtypes = [ctypes.c_char_p]
    lib.axon_stop_nrt_profile.restype = ctypes.c_int64

    @contextlib.contextmanager
    def _hook(output_dir, device_ids):
        import jax
        jax.devices()
        if device_ids:
            ids = (ctypes.c_int64 * len(device_ids))(*device_ids)
            rc = lib.axon_start_nrt_profile(ids, len(device_ids))
        else:
            rc = lib.axon_start_nrt_profile(None, 0)
        if rc != 0:
            raise RuntimeError(f"axon_start_nrt_profile rc={rc}")
        try:
            yield
        finally:
            n = lib.axon_stop_nrt_profile(str(output_dir).encode())
            print(f"profile: {n} file(s) written to {output_dir}")

    mod = types.ModuleType("antenv.axon_hooks")
    mod.get_axon_ntff_profile_hook = lambda: _hook
    mod.set_axon_ntff_profile_hook = lambda h: None
    sys.modules["antenv.axon_hooks"] = mod

    import concourse.bass_utils as bu
    bu.upload_artifacts = lambda tmpdir: tmpdir


def kernel(x, Wq, Wkv, Wout, b_out, _profile=False):
    import sys
    if "/opt/trn_rl_repo" not in sys.path:
        sys.path.insert(0, "/opt/trn_rl_repo")
    from concourse.bass_utils import run_bass_kernel_spmd
    if _profile:
        _install_profile_hook()

    if "nc" not in _cache:
        _cache["nc"] = _build_program()
    nc = _cache["nc"]
    in_maps = _host_inputs(
        np.asarray(x), np.asarray(Wq), np.asarray(Wkv), np.asarray(Wout)
    )
    kwargs = {}
    if _profile:
        import tempfile
        kwargs["tmpdir"] = tempfile.mkdtemp(prefix="bass_prof_")
    res = run_bass_kernel_spmd(
        nc, in_maps, core_ids=list(range(H)), trace=_profile, **kwargs
    )
    y = np.zeros((B * NB * 4, P, 4 * FB), np.float32)
    for rmap in res.results:
        y += rmap["y"].astype(np.float32)
    # unpack [(b, ib, ot4), p, (o, f)] -> [B, CIN, N]
    y = (
        y.reshape(B, NB, 4, P, 4, FB)
        .transpose(0, 2, 4, 3, 1, 5)        # b, ot4, o, p, ib, f
        .reshape(B, CIN, N)
    )
    y = y + np.asarray(b_out)[None, :, None]
    if _profile:
        _cache["last_exec_time_ns"] = res.exec_time_ns
        _cache["last_profile_dir"] = kwargs.get("tmpdir")
        _cache["last_results"] = res
    return y
